# revision 12
# baseline (speedup 1.0000x reference)
"""AttentionContext kernel for Trainium2, data-parallel over batch on 8 cores.

Reference computation (B=64, T=2048, D=512 everywhere):
    phi_s = s @ phi_w.T + phi_b                  # [B, D]
    psi_h = einsum('bth,ah->bta', h, psi_w) + psi_b
    e     = einsum('ba,bta->bt', phi_s, psi_h)   # [B, T]
    alpha = softmax(e, axis=-1)
    c     = alpha * h.sum(-1)                    # [B, T]

Algebraic restructuring:
    e[b,t] = (phi_s[b] @ psi_w) . h[b,t] + const(b)   (const dropped: softmax
    is shift-invariant).  w[b] = phi_s[b] @ psi_w.

v6 structure (per core, 8 batches):
  - stage-0 latency: fp32 PE matmuls are 2-pass (LOW_HIGH), so the big
    512x512 weight product is replaced by a short chain of transposes and
    thin matmuls: phi_w.T via 16 PE transposes (pipelined with the weight
    DMA), phi_s = sT @ phi_wT (4 MMs), transpose phi_s (tiny), then
    w = phi_sT @ psi_w with the phi_b term folded as an early
    phib_rep @ psi_w accumulation.  Per-batch broadcast MMs write separate
    w_bc tiles so the first e-tile only waits for batch 0's broadcast.
  - per h tile [128t, 512d]: e via DVE scalar_tensor_tensor accumulate
    (accum-bearing 2-src DVE ops are 1x by ISA, ~740ns/tile); hsum split
    between DVE tensor_scalar+accum (keeps 2x_2p perf mode, ~410ns/tile -
    front-loaded into DVE's wait-for-w window plus a small tail share) and
    ACT activation-accumulate (~900ns/tile) for the rest.
  - ACT queue order matters (strict FIFO): the w_bc PSUM->SBUF copies are
    interleaved into the hsum stream just-in-time so they don't head-of-
    line-block the early hsum tiles.
  - softmax: exp uses per-partition column-max as bias; the cross-partition
    correction runs via two GpSimd partition_all_reduce ops, software-
    pipelined across batches.
"""

import numpy as np

import concourse.bass as bass
import concourse.bacc as bacc
import concourse.tile as tile
from concourse import mybir
from concourse import bass_isa
from concourse import bass_utils
from concourse.masks import make_identity

FP = mybir.dt.float32
ALU = mybir.AluOpType
AF = mybir.ActivationFunctionType
RED = bass_isa.ReduceOp

N_CORES = 8
B_LOC = 8          # batches per core
T = 2048
D = 512
P = 128
KC = D // P        # 4 contraction chunks of 128
TI = T // P        # 16 t-tiles per batch
SUP = 8            # t-tiles per DMA super-tile
NSUP = TI // SUP   # 2 super-tiles per batch
NST = B_LOC * NSUP
HBUFS = 8          # h super-tile buffers in flight (8 * 2MB = 16MB SBUF)

# hsum tiles per super-tile handled by DVE (tensor_scalar+accum at 2x mode,
# ~410ns/tile): front-loaded into DVE's wait-for-w window, plus a small
# tail share so ACT's queue drains before the final softmax stages.
HD = [8, 8, 4] + [0] * (NST - 4) + [4]


def _emit(nc, tc, variant="full"):
    s = nc.dram_tensor("s", [B_LOC, D], FP, kind="ExternalInput").ap()
    h = nc.dram_tensor("h", [B_LOC, T, D], FP, kind="ExternalInput").ap()
    phi_w = nc.dram_tensor("phi_w", [D, D], FP, kind="ExternalInput").ap()
    phi_b = nc.dram_tensor("phi_b", [D], FP, kind="ExternalInput").ap()
    psi_w = nc.dram_tensor("psi_w", [D, D], FP, kind="ExternalInput").ap()
    c_out = nc.dram_tensor("c", [B_LOC, T], FP, kind="ExternalOutput").ap()

    with (
        tc.tile_pool(name="consts", bufs=1) as consts,
        tc.tile_pool(name="hpool", bufs=HBUFS) as hpool,
        tc.tile_pool(name="psA", bufs=1, space="PSUM") as psA,
        tc.tile_pool(name="psB", bufs=1, space="PSUM") as psB,
        tc.tile_pool(name="junk", bufs=2) as junk,
        tc.tile_pool(name="small", bufs=4) as small,
    ):
        # ------------- input DMAs: weights ride the scalar HWDGE queue ----
        # so the h super-tile stream starts immediately on the sync queue.
        phi_w_sb = consts.tile([P, KC, D], FP)   # [a % 128, a // 128, k]
        psi_w_sb = consts.tile([P, KC, D], FP)   # [a % 128, a // 128, m]
        s_sb = consts.tile([B_LOC, D], FP)
        nc.scalar.dma_start(out=s_sb, in_=s)
        phi_b_sb = consts.tile([P, KC], FP)      # [a % 128, a // 128]
        nc.scalar.dma_start(
            out=phi_b_sb, in_=phi_b.rearrange("(ac p) -> p ac", p=P)
        )
        nc.scalar.dma_start(
            out=phi_w_sb, in_=phi_w.rearrange("(ac p) k -> p ac k", p=P)
        )
        nc.scalar.dma_start(
            out=psi_w_sb, in_=psi_w.rearrange("(ac p) k -> p ac k", p=P)
        )

        ident = consts.tile([P, P], FP)
        make_identity(nc, ident)

        # Warm the ACT exp table set early so the ~2.7us load overlaps.
        tiny = consts.tile([1, 1], FP)
        nc.vector.memset(tiny, 0.0)
        nc.scalar.activation(out=tiny, in_=tiny, func=AF.Exp)

        # phi_b replicated along a free b-axis: [P, KC, B_LOC]
        phib_rep = consts.tile([P, KC, B_LOC], FP)
        pb = phi_b_sb[:, :]
        nc.vector.tensor_copy(
            out=phib_rep,
            in_=bass.AP(
                tensor=pb.tensor,
                offset=pb.offset,
                ap=[[pb.ap[0][0], P], [pb.ap[-1][0], KC], [0, B_LOC]],
            ),
        )

        sT_sb = consts.tile([P, KC, B_LOC], FP)   # s.T[k, b]
        pwt_sb = consts.tile([P, KC, D], FP)      # phi_w.T[k, a], k=kc*128+p
        phis_sb = consts.tile([B_LOC, D], FP)     # phi_s[b, a] (no bias)
        pst_sb = consts.tile([P, KC, B_LOC], FP)  # phi_s.T[a, b]
        w_sb = consts.tile([B_LOC, D], FP)        # w[b, m]
        w_bcs = [
            consts.tile([P, D], FP, name=f"w_bc{b}") for b in range(B_LOC)
        ]                                         # w[b] broadcast down parts
        e_all = consts.tile([P, P], FP)           # e[t%128, b*16 + ti]
        exp_all = consts.tile([P, P], FP)         # exp(e - colmax), same layout
        c_acc = consts.tile([P, B_LOC, TI], FP)   # staged outputs

        # hsum accumulators live in PSUM: ScalarE sits closer to PSUM, so
        # the per-op ACTIVATION_READ_ACCUMULATOR drain is cheaper there.
        hs_all = psB.tile([P, P], FP, tag="pD", name="hs_all")

        # ---------------- stage 0: compute w and broadcast -----------------
        # s.T chunks via PE transpose
        st_ps = psB.tile([P, KC, B_LOC], FP, tag="pA", name="st_ps")
        for kc in range(KC):
            nc.tensor.transpose(
                st_ps[:, kc, :],
                in_=s_sb[:, kc * P : (kc + 1) * P],
                identity=ident[:B_LOC, :B_LOC],
            )
        nc.scalar.copy(out=sT_sb, in_=st_ps)

        # phi_w.T via 16 PE transposes, pipelined with chunk arrivals.
        pwt_ps = [
            psA.tile([P, KC, P], FP, tag=f"pwt{kc}", name=f"pwt_ps{kc}")
            for kc in range(KC)
        ]
        for ac in range(KC):
            for kc in range(KC):
                nc.tensor.transpose(
                    pwt_ps[kc][:, ac, :],
                    in_=phi_w_sb[:, ac, kc * P : (kc + 1) * P],
                    identity=ident,
                )
            if ac == KC - 1:
                for kc in range(KC):
                    nc.scalar.copy(out=pwt_sb[:, kc, :], in_=pwt_ps[kc])

        # phi_s[b, a] = sum_k s[b, k] phi_w[a, k]  (bias folded at w)
        ps_ps = psB.tile([B_LOC, D], FP, tag="pB", name="ps_ps")
        for kc in range(KC):
            nc.tensor.matmul(
                ps_ps,
                lhsT=sT_sb[:, kc, :],
                rhs=pwt_sb[:, kc, :],
                start=(kc == 0),
                stop=(kc == KC - 1),
            )
        nc.scalar.copy(out=phis_sb, in_=ps_ps)

        # phi_s.T via 4 tiny PE transposes
        pst_ps = psB.tile([P, KC, B_LOC], FP, tag="pA", name="pst_ps")
        for ac in range(KC):
            nc.tensor.transpose(
                pst_ps[:, ac, :],
                in_=phis_sb[:, ac * P : (ac + 1) * P],
                identity=ident[:B_LOC, :B_LOC],
            )
        nc.scalar.copy(out=pst_sb, in_=pst_ps)

        # w[b, m] = sum_a (phi_s[b,a] + phi_b[a]) psi_w[a, m].  The phi_b
        # term accumulates first (ready early, off the critical path).
        w_ps = psB.tile([B_LOC, D], FP, tag="pC", name="w_ps")
        for ac in range(KC):
            nc.tensor.matmul(
                w_ps,
                lhsT=phib_rep[:, ac, :],
                rhs=psi_w_sb[:, ac, :],
                start=(ac == 0),
                stop=False,
            )
        for ac in range(KC):
            nc.tensor.matmul(
                w_ps,
                lhsT=pst_sb[:, ac, :],
                rhs=psi_w_sb[:, ac, :],
                start=False,
                stop=(ac == KC - 1),
            )
        nc.scalar.copy(out=w_sb, in_=w_ps)

        # broadcast each w row down the partitions, one PE op per b.  The
        # PSUM->SBUF copies are NOT emitted here: they interleave into
        # ACT's main-loop stream (strict FIFO) just-in-time.
        rowsel = consts.tile([B_LOC, B_LOC, P], FP)
        ident_bc = bass.AP(
            tensor=ident.tensor,
            offset=ident.offset,
            ap=[[ident.ap[0][0], B_LOC], [ident.ap[-1][0], B_LOC], [0, P]],
        )
        nc.vector.tensor_copy(out=rowsel, in_=ident_bc)
        bc_pss = []
        for b in range(B_LOC):
            bc_ps = psA.tile([P, D], FP, tag=f"pwt{b % 4}", name=f"bc{b}")
            nc.tensor.matmul(bc_ps, lhsT=rowsel[:, b, :], rhs=w_sb)
            bc_pss.append(bc_ps)

        if variant == "s0":
            nc.scalar.copy(out=w_bcs[0], in_=bc_pss[0])
            nc.sync.dma_start(out=c_out[:, :D], in_=w_bcs[0][:B_LOC, :])
            return

        # ---------------- stream h; softmax pipelined across batches -------
        state = {}

        def part_a_dve(b):
            cols = slice(b * TI, (b + 1) * TI)
            colmax = small.tile([P, 1], FP, tag="colmax")
            nc.vector.tensor_reduce(
                out=colmax, in_=e_all[:, cols], axis=mybir.AxisListType.X,
                op=ALU.max,
            )
            nbc = small.tile([P, 1], FP, tag="nbc")
            nc.vector.tensor_scalar_mul(out=nbc, in0=colmax, scalar1=-1.0)
            mb = small.tile([P, 1], FP, tag="mb")
            nc.gpsimd.partition_all_reduce(
                out_ap=mb, in_ap=colmax, channels=P, reduce_op=RED.max
            )
            state[b] = (colmax, nbc, mb)

        def part_a_act(b):
            colmax, nbc, mb = state[b]
            cols = slice(b * TI, (b + 1) * TI)
            pscol = small.tile([P, 1], FP, tag="pscol")
            nc.scalar.activation(
                out=exp_all[:, cols],
                in_=e_all[:, cols],
                func=AF.Exp,
                bias=nbc,
                scale=1.0,
                accum_out=pscol,
            )
            state[b] = (colmax, mb, pscol)

        def part_b1(b):
            colmax, mb, pscol = state[b]
            dcm = small.tile([P, 1], FP, tag="dcm")
            nc.vector.tensor_tensor(
                out=dcm, in0=colmax, in1=mb, op=ALU.subtract
            )
            tcor = small.tile([P, 1], FP, tag="tcor")
            nc.scalar.activation(out=tcor, in_=dcm, func=AF.Exp)
            sv = small.tile([P, 1], FP, tag="sv")
            nc.vector.tensor_tensor(out=sv, in0=pscol, in1=tcor, op=ALU.mult)
            sb = small.tile([P, 1], FP, tag="sb")
            nc.gpsimd.partition_all_reduce(
                out_ap=sb, in_ap=sv, channels=P, reduce_op=RED.add
            )
            state[b] = (tcor, sb)

        def part_b2(b):
            tcor, sb = state.pop(b)
            cols = slice(b * TI, (b + 1) * TI)
            rs = small.tile([P, 1], FP, tag="rs")
            nc.vector.reciprocal(out=rs, in_=sb)
            tc2 = small.tile([P, 1], FP, tag="tc2")
            nc.vector.tensor_tensor(out=tc2, in0=tcor, in1=rs, op=ALU.mult)
            nc.vector.scalar_tensor_tensor(
                out=c_acc[:, b, :],
                in0=exp_all[:, cols],
                scalar=tc2,
                in1=hs_all[:, cols],
                op0=ALU.mult,
                op1=ALU.mult,
            )
            # store batch b: t = j*SUP*P + p*SUP + jt
            nc.sync.dma_start(
                out=c_out[b, :].rearrange("(j p jt) -> p j jt", p=P, jt=SUP),
                in_=c_acc[:, b, :],
            )

        def emit_e(st, ht, b, col0):
            for jt in range(SUP):
                jd = junk.tile([P, D], FP, tag="jd")
                nc.vector.scalar_tensor_tensor(
                    out=jd,
                    in0=ht[:, jt, :],
                    scalar=1.0,
                    in1=w_bcs[b],
                    op0=ALU.mult,
                    op1=ALU.mult,
                    accum_out=e_all[:, col0 + jt : col0 + jt + 1],
                )

        def emit_hsum_dve(st, ht, col0):
            for jt in range(HD[st]):
                jh = junk.tile([P, D], FP, tag="jh")
                nc.vector.tensor_scalar(
                    out=jh,
                    in0=ht[:, jt, :],
                    scalar1=1.0,
                    scalar2=0.0,
                    op0=ALU.mult,
                    op1=ALU.add,
                    accum_out=hs_all[:, col0 + jt : col0 + jt + 1],
                )

        def emit_hsum_act(st, ht, col0):
            for jt in range(HD[st], SUP):
                ja = junk.tile([P, D], FP, tag="ja")
                nc.scalar.activation(
                    out=ja,
                    in_=ht[:, jt, :],
                    func=AF.Copy,
                    accum_out=hs_all[:, col0 + jt : col0 + jt + 1],
                )

        hts = []
        for st in range(NST):
            b, j = divmod(st, NSUP)
            ht = hpool.tile([P, SUP, D], FP, tag="ht")
            nc.sync.dma_start(
                out=ht,
                in_=h[b, j * SUP * P : (j + 1) * SUP * P, :].rearrange(
                    "(p jt) d -> p jt d", p=P
                ),
            )
            hts.append(ht)

        # front-loaded DVE hsum tiles run while DVE waits for w_bc[0]
        for st in range(NST - 1):
            if HD[st] > 0:
                b, j = divmod(st, NSUP)
                emit_hsum_dve(st, hts[st], b * TI + j * SUP)

        for st in range(NST):
            b, j = divmod(st, NSUP)
            ht = hts[st]
            col0 = b * TI + j * SUP
            emit_hsum_act(st, ht, col0)
            if st < B_LOC:
                # w_bc[st] PSUM->SBUF copy, just-in-time in ACT's stream
                nc.scalar.copy(out=w_bcs[st], in_=bc_pss[st])
            if st == NST - 1 and HD[st] > 0:
                emit_hsum_dve(st, ht, col0)  # tail share on DVE
            emit_e(st, ht, b, col0)
            if variant == "s1":
                continue
            # pipelined softmax stages, each one supertile apart
            if j == NSUP - 1:
                part_a_dve(b)
                if b >= 1:
                    part_b1(b - 1)
            else:
                if b >= 1:
                    part_a_act(b - 1)
                if b >= 2:
                    part_b2(b - 2)

        if variant == "s1":
            nc.sync.dma_start(out=c_out[:, :P], in_=e_all)
            nc.sync.dma_start(out=c_out[:, P : 2 * P], in_=hs_all)
            return

        part_a_act(B_LOC - 1)
        part_b1(B_LOC - 1)
        part_b2(B_LOC - 2)
        part_b2(B_LOC - 1)


_CACHE = {}


def _build():
    if "nc" not in _CACHE:
        nc = bacc.Bacc(
            "TRN2", target_bir_lowering=False, debug=False, num_devices=N_CORES
        )
        with tile.TileContext(nc) as tc:
            _emit(nc, tc)
        nc.compile()
        _CACHE["nc"] = nc
    return _CACHE["nc"]


def kernel(s, h, phi_w, phi_b, psi_w, psi_b=None, **_unused):
    s = np.ascontiguousarray(np.asarray(s, dtype=np.float32))
    h = np.ascontiguousarray(np.asarray(h, dtype=np.float32))
    phi_w = np.ascontiguousarray(np.asarray(phi_w, dtype=np.float32))
    phi_b = np.ascontiguousarray(np.asarray(phi_b, dtype=np.float32))
    psi_w = np.ascontiguousarray(np.asarray(psi_w, dtype=np.float32))

    nc = _build()
    in_maps = [
        {
            "s": s[i * B_LOC : (i + 1) * B_LOC],
            "h": h[i * B_LOC : (i + 1) * B_LOC],
            "phi_w": phi_w,
            "phi_b": phi_b,
            "psi_w": psi_w,
        }
        for i in range(N_CORES)
    ]
    res = bass_utils.run_bass_kernel_spmd(nc, in_maps, core_ids=list(range(N_CORES)))
    return np.concatenate(
        [res.results[i]["c"] for i in range(N_CORES)], axis=0
    ).astype(np.float32)


# revision 15
# speedup vs baseline: 1.2951x; 1.2951x over previous
"""AttentionContext kernel for Trainium2, data-parallel over batch on 8 cores.

Reference computation (B=64, T=2048, D=512 everywhere):
    phi_s = s @ phi_w.T + phi_b                  # [B, D]
    psi_h = einsum('bth,ah->bta', h, psi_w) + psi_b
    e     = einsum('ba,bta->bt', phi_s, psi_h)   # [B, T]
    alpha = softmax(e, axis=-1)
    c     = alpha * h.sum(-1)                    # [B, T]

Algebraic restructuring:
    e[b,t] = (phi_s[b] @ psi_w) . h[b,t] + const(b)   (const dropped: softmax
    is shift-invariant).  w[b] = phi_s[b] @ psi_w.

v6 structure (per core, 8 batches):
  - stage-0 latency: fp32 PE matmuls are 2-pass (LOW_HIGH), so the big
    512x512 weight product is replaced by a short chain of transposes and
    thin matmuls: phi_w.T via 16 PE transposes (pipelined with the weight
    DMA), phi_s = sT @ phi_wT (4 MMs), transpose phi_s (tiny), then
    w = phi_sT @ psi_w with the phi_b term folded as an early
    phib_rep @ psi_w accumulation.  Per-batch broadcast MMs write separate
    w_bc tiles so the first e-tile only waits for batch 0's broadcast.
  - per h tile [128t, 512d]: e via DVE scalar_tensor_tensor accumulate
    (accum-bearing 2-src DVE ops are 1x by ISA, ~740ns/tile); hsum split
    between DVE tensor_scalar+accum (keeps 2x_2p perf mode, ~410ns/tile -
    front-loaded into DVE's wait-for-w window plus a small tail share) and
    ACT activation-accumulate (~900ns/tile) for the rest.
  - ACT queue order matters (strict FIFO): the w_bc PSUM->SBUF copies are
    interleaved into the hsum stream just-in-time so they don't head-of-
    line-block the early hsum tiles.
  - softmax: exp uses per-partition column-max as bias; the cross-partition
    correction runs via two GpSimd partition_all_reduce ops, software-
    pipelined across batches.
"""

import numpy as np

import concourse.bass as bass
import concourse.bacc as bacc
import concourse.tile as tile
from concourse import mybir
from concourse import bass_isa
from concourse import bass_utils
from concourse.masks import make_identity

FP = mybir.dt.float32
ALU = mybir.AluOpType
AF = mybir.ActivationFunctionType
RED = bass_isa.ReduceOp

N_CORES = 8
B_LOC = 8          # batches per core
T = 2048
D = 512
P = 128
KC = D // P        # 4 contraction chunks of 128
TI = T // P        # 16 t-tiles per batch
SUP = 8            # t-tiles per DMA super-tile
NSUP = TI // SUP   # 2 super-tiles per batch
NST = B_LOC * NSUP
HBUFS = 8          # h super-tile buffers in flight (8 * 2MB = 16MB SBUF)

# hsum tiles per super-tile handled by DVE as ONE grouped tensor_reduce
# (amortized ~540ns/tile, no read-accumulator): st0/st1 front-loaded into
# DVE's wait-for-w window, a trickle mid-stream, double share at the tail
# so ACT's queue drains before the final softmax stages.
HD = [8, 6, 0, 0, 0, 0, 1, 1, 1, 1, 1, 1, 1, 1, 2, 2]


def _emit(nc, tc, variant="full"):
    s = nc.dram_tensor("s", [B_LOC, D], FP, kind="ExternalInput").ap()
    h = nc.dram_tensor("h", [B_LOC, T, D], FP, kind="ExternalInput").ap()
    phi_w = nc.dram_tensor("phi_w", [D, D], FP, kind="ExternalInput").ap()
    phi_b = nc.dram_tensor("phi_b", [D], FP, kind="ExternalInput").ap()
    psi_w = nc.dram_tensor("psi_w", [D, D], FP, kind="ExternalInput").ap()
    c_out = nc.dram_tensor("c", [B_LOC, T], FP, kind="ExternalOutput").ap()

    with (
        tc.tile_pool(name="consts", bufs=1) as consts,
        tc.tile_pool(name="hpool", bufs=HBUFS) as hpool,
        tc.tile_pool(name="psA", bufs=1, space="PSUM") as psA,
        tc.tile_pool(name="psB", bufs=1, space="PSUM") as psB,
        tc.tile_pool(name="junk", bufs=2) as junk,
        tc.tile_pool(name="small", bufs=4) as small,
    ):
        # ------------- input DMAs (sync queue, priority order) ------------
        # Consolidated weight transfers (per-DMA fixed cost ~1.5-2us makes
        # many small weight DMAs the main reason h starts late).  psi_w is
        # only needed at the w-matmuls, so h super-tile 0 jumps ahead of it.
        phi_w_sb = consts.tile([P, KC, D], FP)   # [a % 128, a // 128, k]
        psi_w_sb = consts.tile([P, KC, D], FP)   # [a % 128, a // 128, m]
        nc.sync.dma_start(
            out=phi_w_sb, in_=phi_w.rearrange("(ac p) k -> p ac k", p=P)
        )
        s_sb = consts.tile([B_LOC, D], FP)
        nc.sync.dma_start(out=s_sb, in_=s)
        phi_b_sb = consts.tile([P, KC], FP)      # [a % 128, a // 128]
        nc.sync.dma_start(
            out=phi_b_sb, in_=phi_b.rearrange("(ac p) -> p ac", p=P)
        )
        nc.sync.dma_start(
            out=psi_w_sb, in_=psi_w.rearrange("(ac p) k -> p ac k", p=P)
        )

        ident = consts.tile([P, P], FP)
        make_identity(nc, ident)

        # Warm the ACT exp table set early so the ~2.7us load overlaps.
        tiny = consts.tile([1, 1], FP)
        nc.vector.memset(tiny, 0.0)
        nc.scalar.activation(out=tiny, in_=tiny, func=AF.Exp)

        # phi_b replicated along a free b-axis: [P, KC, B_LOC]
        phib_rep = consts.tile([P, KC, B_LOC], FP)
        pb = phi_b_sb[:, :]
        nc.vector.tensor_copy(
            out=phib_rep,
            in_=bass.AP(
                tensor=pb.tensor,
                offset=pb.offset,
                ap=[[pb.ap[0][0], P], [pb.ap[-1][0], KC], [0, B_LOC]],
            ),
        )

        sT_sb = consts.tile([P, KC, B_LOC], FP)   # s.T[k, b]
        pwt_sb = consts.tile([P, KC, D], FP)      # phi_w.T[k, a], k=kc*128+p
        phis_sb = consts.tile([B_LOC, D], FP)     # phi_s[b, a] (no bias)
        pst_sb = consts.tile([P, KC, B_LOC], FP)  # phi_s.T[a, b]
        w_sb = consts.tile([B_LOC, D], FP)        # w[b, m]
        w_bcs = [
            consts.tile([P, D], FP, name=f"w_bc{b}") for b in range(B_LOC)
        ]                                         # w[b] broadcast down parts
        e_all = consts.tile([P, P], FP)           # e[t%128, b*16 + ti]
        exp_all = consts.tile([P, P], FP)         # exp(e - colmax), same layout
        c_acc = consts.tile([P, B_LOC, TI], FP)   # staged outputs

        hs_all = consts.tile([P, P], FP)          # hsum, same layout

        # ---------------- stage 0: compute w and broadcast -----------------
        # s.T chunks via PE transpose
        st_ps = psB.tile([P, KC, B_LOC], FP, tag="pA", name="st_ps")
        for kc in range(KC):
            nc.tensor.transpose(
                st_ps[:, kc, :],
                in_=s_sb[:, kc * P : (kc + 1) * P],
                identity=ident[:B_LOC, :B_LOC],
            )
        nc.scalar.copy(out=sT_sb, in_=st_ps)

        # phi_w.T via 16 PE transposes, pipelined with chunk arrivals.
        pwt_ps = [
            psA.tile([P, KC, P], FP, tag=f"pwt{kc}", name=f"pwt_ps{kc}")
            for kc in range(KC)
        ]
        for ac in range(KC):
            for kc in range(KC):
                nc.tensor.transpose(
                    pwt_ps[kc][:, ac, :],
                    in_=phi_w_sb[:, ac, kc * P : (kc + 1) * P],
                    identity=ident,
                )
            if ac == KC - 1:
                for kc in range(KC):
                    nc.scalar.copy(out=pwt_sb[:, kc, :], in_=pwt_ps[kc])

        # phi_s[b, a] = sum_k s[b, k] phi_w[a, k]  (bias folded at w)
        ps_ps = psB.tile([B_LOC, D], FP, tag="pB", name="ps_ps")
        for kc in range(KC):
            nc.tensor.matmul(
                ps_ps,
                lhsT=sT_sb[:, kc, :],
                rhs=pwt_sb[:, kc, :],
                start=(kc == 0),
                stop=(kc == KC - 1),
            )
        nc.scalar.copy(out=phis_sb, in_=ps_ps)

        # phi_s.T via 4 tiny PE transposes
        pst_ps = psB.tile([P, KC, B_LOC], FP, tag="pA", name="pst_ps")
        for ac in range(KC):
            nc.tensor.transpose(
                pst_ps[:, ac, :],
                in_=phis_sb[:, ac * P : (ac + 1) * P],
                identity=ident[:B_LOC, :B_LOC],
            )
        nc.scalar.copy(out=pst_sb, in_=pst_ps)

        # w[b, m] = sum_a (phi_s[b,a] + phi_b[a]) psi_w[a, m].  The phi_b
        # term accumulates first (ready early, off the critical path).
        w_ps = psB.tile([B_LOC, D], FP, tag="pC", name="w_ps")
        for ac in range(KC):
            nc.tensor.matmul(
                w_ps,
                lhsT=phib_rep[:, ac, :],
                rhs=psi_w_sb[:, ac, :],
                start=(ac == 0),
                stop=False,
            )
        for ac in range(KC):
            nc.tensor.matmul(
                w_ps,
                lhsT=pst_sb[:, ac, :],
                rhs=psi_w_sb[:, ac, :],
                start=False,
                stop=(ac == KC - 1),
            )
        nc.scalar.copy(out=w_sb, in_=w_ps)

        # broadcast each w row down the partitions, one PE op per b.  The
        # PSUM->SBUF copies are NOT emitted here: they interleave into
        # ACT's main-loop stream (strict FIFO) just-in-time.
        rowsel = consts.tile([B_LOC, B_LOC, P], FP)
        ident_bc = bass.AP(
            tensor=ident.tensor,
            offset=ident.offset,
            ap=[[ident.ap[0][0], B_LOC], [ident.ap[-1][0], B_LOC], [0, P]],
        )
        nc.vector.tensor_copy(out=rowsel, in_=ident_bc)
        bc_pss = []
        for b in range(B_LOC):
            bc_ps = psA.tile([P, D], FP, tag=f"pwt{b % 4}", name=f"bc{b}")
            nc.tensor.matmul(bc_ps, lhsT=rowsel[:, b, :], rhs=w_sb)
            bc_pss.append(bc_ps)

        if variant == "s0":
            nc.scalar.copy(out=w_bcs[0], in_=bc_pss[0])
            nc.sync.dma_start(out=c_out[:, :D], in_=w_bcs[0][:B_LOC, :])
            return

        # ---------------- stream h; softmax pipelined across batches -------
        state = {}

        def part_a_dve(b):
            cols = slice(b * TI, (b + 1) * TI)
            colmax = small.tile([P, 1], FP, tag="colmax")
            nc.vector.tensor_reduce(
                out=colmax, in_=e_all[:, cols], axis=mybir.AxisListType.X,
                op=ALU.max,
            )
            nbc = small.tile([P, 1], FP, tag="nbc")
            nc.vector.tensor_scalar_mul(out=nbc, in0=colmax, scalar1=-1.0)
            mb = small.tile([P, 1], FP, tag="mb")
            nc.gpsimd.partition_all_reduce(
                out_ap=mb, in_ap=colmax, channels=P, reduce_op=RED.max
            )
            state[b] = (colmax, nbc, mb)

        def part_a_act(b):
            colmax, nbc, mb = state[b]
            cols = slice(b * TI, (b + 1) * TI)
            pscol = small.tile([P, 1], FP, tag="pscol")
            nc.scalar.activation(
                out=exp_all[:, cols],
                in_=e_all[:, cols],
                func=AF.Exp,
                bias=nbc,
                scale=1.0,
                accum_out=pscol,
            )
            state[b] = (colmax, mb, pscol)

        def part_b1(b):
            colmax, mb, pscol = state[b]
            dcm = small.tile([P, 1], FP, tag="dcm")
            nc.vector.tensor_tensor(
                out=dcm, in0=colmax, in1=mb, op=ALU.subtract
            )
            tcor = small.tile([P, 1], FP, tag="tcor")
            nc.scalar.activation(out=tcor, in_=dcm, func=AF.Exp)
            sv = small.tile([P, 1], FP, tag="sv")
            nc.vector.tensor_tensor(out=sv, in0=pscol, in1=tcor, op=ALU.mult)
            sb = small.tile([P, 1], FP, tag="sb")
            nc.gpsimd.partition_all_reduce(
                out_ap=sb, in_ap=sv, channels=P, reduce_op=RED.add
            )
            state[b] = (tcor, sb)

        def part_b2(b):
            tcor, sb = state.pop(b)
            cols = slice(b * TI, (b + 1) * TI)
            rs = small.tile([P, 1], FP, tag="rs")
            nc.vector.reciprocal(out=rs, in_=sb)
            tc2 = small.tile([P, 1], FP, tag="tc2")
            nc.vector.tensor_tensor(out=tc2, in0=tcor, in1=rs, op=ALU.mult)
            nc.vector.scalar_tensor_tensor(
                out=c_acc[:, b, :],
                in0=exp_all[:, cols],
                scalar=tc2,
                in1=hs_all[:, cols],
                op0=ALU.mult,
                op1=ALU.mult,
            )
            # store batch b: t = j*SUP*P + p*SUP + jt
            nc.sync.dma_start(
                out=c_out[b, :].rearrange("(j p jt) -> p j jt", p=P, jt=SUP),
                in_=c_acc[:, b, :],
            )

        def emit_e(st, ht, b, col0):
            for jt in range(SUP):
                jd = junk.tile([P, D], FP, tag="jd")
                nc.vector.scalar_tensor_tensor(
                    out=jd,
                    in0=ht[:, jt, :],
                    scalar=1.0,
                    in1=w_bcs[b],
                    op0=ALU.mult,
                    op1=ALU.mult,
                    accum_out=e_all[:, col0 + jt : col0 + jt + 1],
                )

        def emit_hsum_dve(st, ht, col0):
            r = HD[st]
            if r > 0:
                nc.vector.tensor_reduce(
                    out=hs_all[:, col0 : col0 + r],
                    in_=ht[:, :r, :],
                    axis=mybir.AxisListType.X,
                    op=ALU.add,
                )

        def emit_hsum_act(st, ht, col0):
            for jt in range(HD[st], SUP):
                ja = junk.tile([P, D], FP, tag="ja")
                nc.scalar.activation(
                    out=ja,
                    in_=ht[:, jt, :],
                    func=AF.Copy,
                    accum_out=hs_all[:, col0 + jt : col0 + jt + 1],
                )

        hts = []
        for st in range(NST):
            b, j = divmod(st, NSUP)
            ht = hpool.tile([P, SUP, D], FP, tag="ht")
            nc.sync.dma_start(
                out=ht,
                in_=h[b, j * SUP * P : (j + 1) * SUP * P, :].rearrange(
                    "(p jt) d -> p jt d", p=P
                ),
            )
            hts.append(ht)

        # front-loaded DVE hsum groups run while DVE waits for w_bc[0]
        for st in range(2):
            if HD[st] > 0:
                b, j = divmod(st, NSUP)
                emit_hsum_dve(st, hts[st], b * TI + j * SUP)

        for st in range(NST):
            b, j = divmod(st, NSUP)
            ht = hts[st]
            col0 = b * TI + j * SUP
            emit_hsum_act(st, ht, col0)
            if st < B_LOC:
                # w_bc[st] PSUM->SBUF copy, just-in-time in ACT's stream
                nc.scalar.copy(out=w_bcs[st], in_=bc_pss[st])
            if st >= 2 and HD[st] > 0:
                emit_hsum_dve(st, ht, col0)  # mid/tail share on DVE
            emit_e(st, ht, b, col0)
            if variant == "s1":
                continue
            # pipelined softmax stages, each one supertile apart
            if j == NSUP - 1:
                part_a_dve(b)
                if b >= 1:
                    part_b1(b - 1)
            else:
                if b >= 1:
                    part_a_act(b - 1)
                if b >= 2:
                    part_b2(b - 2)

        if variant == "s1":
            nc.sync.dma_start(out=c_out[:, :P], in_=e_all)
            nc.sync.dma_start(out=c_out[:, P : 2 * P], in_=hs_all)
            return

        part_a_act(B_LOC - 1)
        part_b1(B_LOC - 1)
        part_b2(B_LOC - 2)
        part_b2(B_LOC - 1)


_CACHE = {}


def _build():
    if "nc" not in _CACHE:
        nc = bacc.Bacc(
            "TRN2", target_bir_lowering=False, debug=False, num_devices=N_CORES
        )
        with tile.TileContext(nc) as tc:
            _emit(nc, tc)
        nc.compile()
        _CACHE["nc"] = nc
    return _CACHE["nc"]


def kernel(s, h, phi_w, phi_b, psi_w, psi_b=None, **_unused):
    s = np.ascontiguousarray(np.asarray(s, dtype=np.float32))
    h = np.ascontiguousarray(np.asarray(h, dtype=np.float32))
    phi_w = np.ascontiguousarray(np.asarray(phi_w, dtype=np.float32))
    phi_b = np.ascontiguousarray(np.asarray(phi_b, dtype=np.float32))
    psi_w = np.ascontiguousarray(np.asarray(psi_w, dtype=np.float32))

    nc = _build()
    in_maps = [
        {
            "s": s[i * B_LOC : (i + 1) * B_LOC],
            "h": h[i * B_LOC : (i + 1) * B_LOC],
            "phi_w": phi_w,
            "phi_b": phi_b,
            "psi_w": psi_w,
        }
        for i in range(N_CORES)
    ]
    res = bass_utils.run_bass_kernel_spmd(nc, in_maps, core_ids=list(range(N_CORES)))
    return np.concatenate(
        [res.results[i]["c"] for i in range(N_CORES)], axis=0
    ).astype(np.float32)
